# revision 49
# baseline (speedup 1.0000x reference)
"""Trainium2 Bass kernel for a diffusers-style cross-attention block.

Problem (hardcoded shapes):
    hidden_states         [2, 2048, 1280] f32
    encoder_hidden_states [2, 2048, 1024] f32
    Wq [1280, 1280]  Wk/Wv [1024, 1280]  Wo [1280, 1280]  b_o [1280]  (all f32)
    out = softmax((x Wq) (enc Wk)^T / 8) (enc Wv) Wo + b_o      (20 heads x 64)

Sharding across 8 NeuronCores: data-parallel on batch (2) x tensor-parallel on
heads (4 groups of 5 heads). Each core computes a partial output
[2048, 1280] = A_local @ Wo_rows for its 5 heads; the host sums the 4 partials
per batch element and adds the bias.

Per-core layout: the host passes TRANSPOSED activations (x^T, enc^T, bf16), so
Q^T and K^T come straight out of the projection matmuls, scores are computed
as S^T (kv on partitions, q on free) and exp runs on the scalar engine
PSUM->SBUF. The PV matmul uses lhsT=P^T tiles (M=128 q rows) with rhs=V
blocks (N=65 incl. a ones column that emits the softmax denominator), so it
streams only 65 columns per kv-tile -- half the engine time of the M=65/N=512
formulation.  The resulting A [q, d] tiles are normalized (reciprocal of the
l column broadcast per-partition, fused into the PSUM->SBUF drain) and
transposed back to A^T for the output projection with DMA xbar transposes
(SBUF->SBUF, no PE or PSUM cost); the final round uses a PE transpose
instead, since the xbar round-trip would gate the last out-projection rows.

Scheduling: the PV matmuls run one iteration behind the score/exp stream
(lag-1 software pipeline) so the scalar engine never stalls at round
boundaries.  The V projection is split by head-group and interleaved into
round 0 of each attention stage, which balances the tensor engine against
the scalar engine's exp throughput in every round.  Stages run in order t=2
(single head 4), t=0, t=1: the out-projection is gated by the LAST stage's
A tiles, and a pair stage has twice the exp time per round under which to
hide the out-projection matmuls.  x^T lands q-block-major so stage t=2's
first q-block projection unblocks after 1/4 of the x^T bytes (its other
q-blocks stream in later as attention-round fills); the prologue projects
kt2's first kv-block plus all of Kt0 k-outer, each kstep firing as its
enc^T chunk lands, putting the first exp ~23us in instead of ~35us.
"""

import numpy as np
import ml_dtypes
from contextlib import ExitStack

S = 2048          # seq len (q and kv)
C = 1280          # hidden
CC = 1024         # encoder hidden
HG = 5            # heads per core
D = 64            # head dim
HD = HG * D       # 320
VW = D + 1        # V columns incl. ones column
CK = C // 128     # 10
CCK = CC // 128   # 8
NKV = S // 128    # 16
NQ = S // 512     # 4

_CACHED = {}

# scheduling knobs (sweepable; defaults = best known)
CONFIG = {
    "phat_bufs": 12,
    "osb_bufs": 4,
    "small_bufs": 2,
    "f1_group": 3, "f2_group": 2, "og_group": 3,
    "an_bufs": 6,
    "rate0": 1.3, "rate1": 1.0, "rate2": 1.5, "fill_start": 4,
}


def _emit(ctx, tc, xT, encT, wq, wk, wv, wo, out):
    from concourse import mybir, masks

    nc = tc.nc
    bf16, f32 = mybir.dt.bfloat16, mybir.dt.float32
    Exp = mybir.ActivationFunctionType.Exp

    const = ctx.enter_context(tc.tile_pool(name="const", bufs=1))
    acts = ctx.enter_context(tc.tile_pool(name="acts", bufs=1))
    small = ctx.enter_context(tc.tile_pool(name="small", bufs=CONFIG["small_bufs"]))
    osb_pool = ctx.enter_context(tc.tile_pool(name="osb", bufs=CONFIG["osb_bufs"]))
    phat_pool = ctx.enter_context(tc.tile_pool(name="phat", bufs=CONFIG["phat_bufs"]))
    an_pool = ctx.enter_context(tc.tile_pool(name="an", bufs=CONFIG["an_bufs"]))
    psum = ctx.enter_context(tc.tile_pool(name="psum", bufs=2, space="PSUM"))

    # ---- DMA in, critical-path order. enc^T and x^T land one 128-row
    # contraction chunk at a time so the k-outer prologue projections can
    # start each kstep as soon as its chunk arrives. wv's head-4 columns get
    # a tiny early DMA (stage t=2's V projection runs ~2us into round 0);
    # the rest of wv follows x^T.
    wk_sb = const.tile([128, CCK * HD], bf16, tag="wk")
    nc.sync.dma_start(wk_sb[:], wk.rearrange("(k p) d -> p k d", p=128))
    encT_big = acts.tile([128, CCK * S], bf16, tag="encT")
    for k in range(CCK):
        nc.sync.dma_start(encT_big[:, k * S:(k + 1) * S],
                          encT[k * 128:(k + 1) * 128, :])
    encT_sb = [encT_big[:, k * S:(k + 1) * S] for k in range(CCK)]
    wq_sb = const.tile([128, CK * HD], bf16, tag="wq")
    nc.sync.dma_start(wq_sb[:], wq.rearrange("(k p) d -> p k d", p=128))
    wv_sb = const.tile([128, CCK * HD], bf16, tag="wv")
    wv_pkd = wv.rearrange("(k p) d -> p k d", p=128)
    wv_sb_pkd = wv_sb.rearrange("p (k d) -> p k d", k=CCK)
    nc.sync.dma_start(wv_sb_pkd[:, :, 4 * D:5 * D], wv_pkd[:, :, 4 * D:5 * D])
    # x^T lands q-block-major (j-outer, k-pair chunks): stage t=2's round 0
    # only reads q-block 0, so its Q projection unblocks after 1/4 of x^T;
    # j=1..3 projections run as fills inside the attention rounds.
    xT_big = acts.tile([128, CK * S], bf16, tag="xT")
    xT_kv = xT_big.rearrange("p (k s) -> p k s", k=CK)
    for j in range(NQ):
        for kp in range(0, CK, 2):
            nc.sync.dma_start(
                xT_kv[:, kp:kp + 2, j * 512:(j + 1) * 512],
                xT[kp * 128:(kp + 2) * 128, j * 512:(j + 1) * 512]
                .rearrange("(k p) s -> p k s", p=128),
            )
    xT_sb = [xT_big[:, k * S:(k + 1) * S] for k in range(CK)]
    nc.sync.dma_start(wv_sb_pkd[:, :, 0:4 * D], wv_pkd[:, :, 0:4 * D])
    wo_sb = []
    for t in range(3):
        K = 128 if t < 2 else 64
        w = const.tile([128, C], bf16, tag=f"wo{t}", name=f"wo{t}")
        nc.sync.dma_start(w[:K, :], wo[t * 128:t * 128 + K, :])
        wo_sb.append(w)

    ident = const.tile([128, 128], bf16, tag="ident")
    masks.make_identity(nc, ident[:])

    # persistent intermediates (head pairs packed into 128 partitions)
    qt_sb = [acts.tile([128, S], bf16, tag=f"qt{t}", name=f"qt{t}") for t in range(3)]
    kt_sb = [acts.tile([128, S], bf16, tag=f"kt{t}", name=f"kt{t}") for t in range(3)]
    at_sb = [acts.tile([128, S], bf16, tag=f"at{t}", name=f"at{t}") for t in range(3)]
    v_sb = acts.tile([128, NKV * HG * VW], bf16, tag="v")
    nc.vector.memset(v_sb[:], 1.0)  # ones columns; V blocks overwritten below

    def proj_qk_steps(w_sb, src_sb, nk, dst, t, group):
        """Generator: emits the Q/K projection for tile t in ~group-MM slices."""
        M = 128 if t < 2 else 64
        for j in range(NQ):
            ps = psum.tile([128, 512], f32, tag=("s" if j % 2 == 0 else "fill"),
                           name="ps", bufs=2)
            for k in range(nk):
                nc.tensor.matmul(
                    ps[:M, :],
                    lhsT=w_sb[:, k * HD + t * 128: k * HD + t * 128 + M],
                    rhs=src_sb[k][:, j * 512:(j + 1) * 512],
                    start=(k == 0), stop=(k == nk - 1),
                )
                if (k + 1) % group == 0:
                    yield
            nc.vector.tensor_copy(dst[:M, j * 512:(j + 1) * 512], ps[:M, :])
            yield

    def proj_qk(w_sb, src_sb, nk, dst, t):
        for _ in proj_qk_steps(w_sb, src_sb, nk, dst, t, group=999):
            pass

    def proj_v_tile(i, h0, nhv):
        # V projection for heads h0..h0+nhv-1 of kv-tile i (ones cols kept)
        ps = psum.tile([128, 512], f32, tag="fill", name="ps", bufs=2)
        for k in range(CCK):
            nc.tensor.matmul(
                ps[:, :nhv * D],
                lhsT=encT_sb[k][:, i * 128:(i + 1) * 128],
                rhs=wv_sb[:, k * HD + h0 * D: k * HD + (h0 + nhv) * D],
                start=(k == 0), stop=(k == CCK - 1),
            )
        for h in range(h0, h0 + nhv):
            nc.vector.tensor_copy(
                v_sb[:, (i * HG + h) * VW: (i * HG + h) * VW + D],
                ps[:, (h - h0) * D:(h - h0 + 1) * D],
            )

    def proj_qk_fill_steps(w_sb, src_sb, nk, dst, t, group, tag="fill",
                           js=tuple(range(NQ))):
        M = 128 if t < 2 else 64
        for j in js:
            ps = psum.tile([128, 512], f32, tag=tag, name="ps", bufs=2)
            for k in range(nk):
                nc.tensor.matmul(
                    ps[:M, :],
                    lhsT=w_sb[:, k * HD + t * 128: k * HD + t * 128 + M],
                    rhs=src_sb[k][:, j * 512:(j + 1) * 512],
                    start=(k == 0), stop=(k == nk - 1),
                )
                if (k + 1) % group == 0:
                    yield
            nc.vector.tensor_copy(dst[:M, j * 512:(j + 1) * 512], ps[:M, :])
            yield

    def proj_k_outer_steps(w_sb, src_sb, nk, dst, t, js=tuple(range(NQ)),
                           tag=None, tags=None):
        """k-outer projection over the q-blocks in js: one kstep covers all
        of them, so kstep k can start as soon as src chunk k's DMA lands.
        Yields per kstep."""
        M = 128 if t < 2 else 64
        if tags is None:
            tags = [(tag or ("s" if ji % 2 == 0 else "fill"))
                    for ji in range(len(js))]
        tiles = {j: psum.tile([128, 512], f32, tag=tags[ji],
                              name="pko", bufs=2) for ji, j in enumerate(js)}
        for k in range(nk):
            for j in js:
                nc.tensor.matmul(
                    tiles[j][:M, :],
                    lhsT=w_sb[:, k * HD + t * 128: k * HD + t * 128 + M],
                    rhs=src_sb[k][:, j * 512:(j + 1) * 512],
                    start=(k == 0), stop=(k == nk - 1),
                )
                if k == nk - 1:
                    # copy each j the moment its chain stops -- j=0's copy
                    # gates the first scores of the following stage
                    nc.vector.tensor_copy(dst[:M, j * 512:(j + 1) * 512],
                                          tiles[j][:M, :])
            yield

    def outproj_steps(ms, group, tail=False):
        for m in ms:
            osb = osb_pool.tile([128, C], f32, tag="osb", name="osb")
            cnt = 0
            for ci, c0 in enumerate(range(0, C, 512)):
                cn = min(512, C - c0)
                ps = psum.tile([128, 512], f32,
                               tag=("s" if tail and (m * 3 + ci) % 2 else "fill"),
                               name="ops", bufs=2)
                # t=1 (the last stage's rows, the gating input) contracts
                # last so the other partials can run before at[1] lands
                for ti, t in enumerate((0, 2, 1)):
                    K = 128 if t < 2 else 64
                    nc.tensor.matmul(
                        ps[:, :cn],
                        lhsT=at_sb[t][:K, m * 128:(m + 1) * 128],
                        rhs=wo_sb[t][:K, c0:c0 + cn],
                        start=(ti == 0), stop=(ti == 2),
                    )
                    cnt += 1
                    if cnt % group == 0:
                        yield
                # in the tail the scalar engine is idle -- alternate the
                # psum->sbuf drains across it and DVE
                if tail and (m + ci) % 2:
                    nc.scalar.copy(osb[:, c0:c0 + cn], ps[:, :cn])
                else:
                    nc.vector.tensor_copy(osb[:, c0:c0 + cn], ps[:, :cn])
                nc.sync.dma_start(out[m * 128:(m + 1) * 128, c0:c0 + cn],
                                  osb[:, c0:c0 + cn])
            yield

    def attention(t, v_interleave=False, fills=(None, None, None, None),
                  fill_rate=1.0, fill_start=0, tail_stage=False):
        """Four 512-wide q-block rounds; fills[r] is a generator driven during
        round r (must only read data produced in rounds < r). Pair rounds pack
        both heads side-by-side in one [128,1024] score tile -> single exp.
        fill_rate may be fractional (yields per iteration, spread evenly);
        fill_start delays drives by that many iterations -- needed when the
        fill reads at_sb written by the deferred transpose flush.

        The PV matmuls run one iteration behind the score/exp stream (lag-1
        software pipeline) so the next round's scores are already queued on
        the PE when it waits for the current exp -- the scalar engine never
        stalls at round boundaries."""
        heads = (2 * t, 2 * t + 1) if t < 2 else (4,)
        nh = len(heads)
        h0v = heads[0]
        M = 128 if nh == 2 else 64
        istep = 2 // nh
        state = {"pv": None, "prev": None}

        def norm_round(jb_, pvd):
            # normalize A rows by the denominator column, packing head pairs
            # side-by-side; recips first, then muls qt-major so an[qt]
            # completes early in the DVE stream. The A -> A^T transpose is a
            # DMA xbar transpose (SBUF->SBUF, ~14ns per 16x128 tile) issued
            # from the SP queue: no PE time, no PSUM banks. For the
            # single-head stage the unused half of an is zeroed once so the
            # full 128-wide tile is transposable (at rows 64+ are unused).
            an = [an_pool.tile([128, 128], bf16, tag="an", name="an",
                               bufs=CONFIG["an_bufs"]) for _ in range(4)]
            linvs = {}
            for h in heads:
                linv = small.tile([128, 4], f32, tag="linv", name="linv")
                nc.vector.reciprocal(linv[:], pvd[h][:, D::VW])
                linvs[h] = linv
            tail_round = tail_stage and jb_ == NQ - 1
            for qt in range(4):
                if nh == 1:
                    nc.vector.memset(an[qt][:, 64:128], 0.0)
                for h in heads:
                    rb0 = (h % 2) * 64
                    if tail_round and h % 2:
                        # scalar engine is idle after the last exp; splitting
                        # the normalization muls halves the at-ready latency
                        nc.scalar.activation(
                            an[qt][:, rb0:rb0 + 64],
                            pvd[h][:, qt * VW: qt * VW + D],
                            mybir.ActivationFunctionType.Copy,
                            scale=linvs[h][:, qt:qt + 1],
                        )
                    else:
                        nc.vector.tensor_scalar_mul(
                            an[qt][:, rb0:rb0 + 64],
                            pvd[h][:, qt * VW: qt * VW + D],
                            linvs[h][:, qt:qt + 1],
                        )
                if tail_round:
                    # tail: the score pipeline is done, PSUM banks are free
                    # and the xbar round-trip (~2.5us) would gate the last
                    # out-projection rows -- use the PE transpose instead
                    tp = psum.tile([128, 128], bf16, tag="s", name="tp", bufs=2)
                    nc.tensor.transpose(tp[:, :], an[qt][:, :], ident[:])
                    nc.vector.tensor_copy(
                        at_sb[t][:, jb_ * 512 + qt * 128: jb_ * 512 + (qt + 1) * 128],
                        tp[:, :],
                    )
                else:
                    nc.sync.dma_start_transpose(
                        at_sb[t][:, jb_ * 512 + qt * 128: jb_ * 512 + (qt + 1) * 128],
                        an[qt][:, :],
                    )

        def emit_prev_pv():
            # PV in A-layout: lhsT = P^T q-tile (M=128 q rows), rhs = V
            # block (N=65 cols incl. the denominator ones column). PSUM
            # zeroing is per 2KB bank, so the whole pv tile is one
            # accumulation group: start only on the first touch of the
            # bank, stop on the last; intermediate first-touches overwrite
            # their still-pending-zero bytes.
            if state["prev"] is None:
                return
            ph_, i0_, jb_ = state["prev"]
            state["prev"] = None
            if i0_ == 0:
                state["pv"] = {
                    h: psum.tile([128, 4 * VW], f32, tag="pv", name="pv", bufs=2)
                    for h in heads
                }
            pvd = state["pv"]
            for sx in range(2):
                h = heads[sx % nh]
                i = i0_ + sx // nh
                for qt in range(4):
                    nc.tensor.matmul(
                        pvd[h][:, qt * VW:(qt + 1) * VW],
                        lhsT=ph_[:, sx * 512 + qt * 128: sx * 512 + (qt + 1) * 128],
                        rhs=v_sb[:, (i * HG + h) * VW: (i * HG + h + 1) * VW],
                        start=(i == 0 and qt == 0),
                        stop=(i == NKV - 1 and qt == 3),
                    )
            if i0_ == NKV - istep:
                norm_round(jb_, pvd)

        nits = NKV // istep
        for jb in range(NQ):
            fill = fills[jb]
            acc = 0.0
            # pair rounds pack (headA | headB) per kv-tile; single-head
            # rounds pack (kv-tile i | kv-tile i+1) -- one exp per 1024 cols
            for it, i0 in enumerate(range(0, NKV, istep)):
                sps = psum.tile([128, 1024], f32, tag="s", name="sps", bufs=2)
                for sx in range(2):
                    h = heads[sx % nh]
                    i = i0 + sx // nh
                    rb0 = (h % 2) * 64
                    nc.tensor.matmul(
                        sps[:, sx * 512:(sx + 1) * 512],
                        lhsT=kt_sb[t][rb0:rb0 + 64, i * 128:(i + 1) * 128],
                        rhs=qt_sb[t][rb0:rb0 + 64, jb * 512:(jb + 1) * 512],
                        start=True, stop=True,
                    )
                ph = phat_pool.tile([128, 1024], bf16, tag="ph", name="ph")
                nc.scalar.activation(ph[:], sps[:], Exp, scale=0.125)
                emit_prev_pv()
                if v_interleave and jb == 0:
                    for i in range(i0, i0 + istep):
                        proj_v_tile(i, h0v, nh)
                if fill is not None and fill_start <= it < nits - 2:
                    acc += fill_rate
                    while acc >= 1.0:
                        next(fill, None)
                        acc -= 1.0
                state["prev"] = (ph, i0, jb)
        emit_prev_pv()

    def chain(*gens):
        for g in gens:
            yield from g

    def drain(g):
        for _ in g:
            pass

    # prologue pass A (paced by the enc^T chunk DMAs): kt2's first kv-block
    # -- the only one the first two score iterations read -- plus ALL of Kt0,
    # zipped kstep-for-kstep on the same chunk deps across the s/pv/fill psum
    # tags. kt2's remaining kv-blocks follow during the x^T j0 DMA window,
    # then Q-t2 j0 streams in chunk-chained with nothing in its way.
    genKj0 = proj_k_outer_steps(wk_sb, encT_sb, CCK, kt_sb[2], 2,
                                js=(0,), tags=("s",))
    genk0a = proj_k_outer_steps(wk_sb, encT_sb, CCK, kt_sb[0], 0,
                                js=(0, 1), tags=("pv", "pv"))
    genk0b = proj_k_outer_steps(wk_sb, encT_sb, CCK, kt_sb[0], 0,
                                js=(2, 3), tags=("fill", "fill"))
    for _ in genKj0:
        next(genk0a, None)
        next(genk0b, None)
    drain(genk0a)
    drain(genk0b)
    drain(proj_k_outer_steps(wk_sb, encT_sb, CCK, kt_sb[2], 2,
                             js=(1, 2, 3), tags=("s", "fill", "s")))
    drain(proj_qk_fill_steps(wq_sb, xT_sb, CK, qt_sb[2], 2,
                             CONFIG["f1_group"], tag="s", js=(0,)))

    fill1 = chain(
        proj_qk_fill_steps(wq_sb, xT_sb, CK, qt_sb[2], 2, CONFIG["f1_group"], js=(1,)),
        proj_qk_fill_steps(wq_sb, xT_sb, CK, qt_sb[2], 2, CONFIG["f1_group"], js=(2,)),
        proj_qk_fill_steps(wq_sb, xT_sb, CK, qt_sb[2], 2, CONFIG["f1_group"], js=(3,)),
        proj_qk_fill_steps(wq_sb, xT_sb, CK, qt_sb[0], 0, CONFIG["f1_group"]),
    )
    attention(2, v_interleave=True, fills=(fill1, fill1, fill1, fill1),
              fill_rate=CONFIG["rate0"], fill_start=2)
    fill2 = chain(fill1,
                  proj_qk_fill_steps(wk_sb, encT_sb, CCK, kt_sb[1], 1, CONFIG["f2_group"]),
                  proj_qk_fill_steps(wq_sb, xT_sb, CK, qt_sb[1], 1, CONFIG["f2_group"]))
    attention(0, v_interleave=True, fills=(None, fill2, fill2, fill2),
              fill_rate=CONFIG["rate1"])
    drain(fill2)
    # out-projection row-group k (m = 4k..4k+3) reads at columns produced by
    # round k of every stage; the last stage (t=1) writes them at iteration 0
    # of round k+1 via the norm + xbar-transpose path, so the og chain may
    # only be driven from fill_start iterations in, and at a rate that keeps
    # cumulative drives by end of round r within r*20 yields (og gating).
    og = [outproj_steps(range(4 * k, 4 * k + 4), CONFIG["og_group"]) for k in range(3)]
    ogc = chain(*og)
    attention(1, v_interleave=True, fills=(None, ogc, ogc, ogc),
              fill_rate=CONFIG["rate2"], fill_start=CONFIG["fill_start"],
              tail_stage=True)
    drain(ogc)
    drain(outproj_steps(range(12, NKV), 999, tail=True))


def build():
    if "nc" in _CACHED:
        return _CACHED["nc"]
    import concourse.tile as tile
    from concourse import bacc, mybir

    bf16, f32 = mybir.dt.bfloat16, mybir.dt.float32
    nc = bacc.Bacc("TRN2", target_bir_lowering=False, debug=False)
    xT = nc.dram_tensor("xT", [C, S], bf16, kind="ExternalInput").ap()
    encT = nc.dram_tensor("encT", [CC, S], bf16, kind="ExternalInput").ap()
    wq = nc.dram_tensor("wq", [C, HD], bf16, kind="ExternalInput").ap()
    wk = nc.dram_tensor("wk", [CC, HD], bf16, kind="ExternalInput").ap()
    wv = nc.dram_tensor("wv", [CC, HD], bf16, kind="ExternalInput").ap()
    wo = nc.dram_tensor("wo", [HD, C], bf16, kind="ExternalInput").ap()
    out = nc.dram_tensor("out", [S, C], f32, kind="ExternalOutput").ap()

    with tile.TileContext(nc) as tc:
        with ExitStack() as ctx:
            _emit(ctx, tc, xT, encT, wq, wk, wv, wo, out)
    nc.compile()
    _CACHED["nc"] = nc
    return nc


def make_in_maps(hidden_states, encoder_hidden_states, Wq, Wk, Wv, Wo):
    bf = ml_dtypes.bfloat16
    in_maps = []
    xTs = [np.ascontiguousarray(hidden_states[b].T).astype(bf) for b in range(2)]
    encTs = [np.ascontiguousarray(encoder_hidden_states[b].T).astype(bf) for b in range(2)]
    for core in range(8):
        b, g = divmod(core, 4)
        cols = slice(g * HD, (g + 1) * HD)
        in_maps.append({
            "xT": xTs[b],
            "encT": encTs[b],
            "wq": np.ascontiguousarray(Wq[:, cols]).astype(bf),
            "wk": np.ascontiguousarray(Wk[:, cols]).astype(bf),
            "wv": np.ascontiguousarray(Wv[:, cols]).astype(bf),
            "wo": np.ascontiguousarray(Wo[cols, :]).astype(bf),
        })
    return in_maps


def kernel(hidden_states, encoder_hidden_states, Wq, Wk, Wv, Wo, b_o):
    from concourse.bass_utils import run_bass_kernel_spmd

    nc = build()
    in_maps = make_in_maps(hidden_states, encoder_hidden_states, Wq, Wk, Wv, Wo)
    res = run_bass_kernel_spmd(nc, in_maps, core_ids=list(range(8)))
    outs = [res.results[c]["out"] for c in range(8)]
    full = np.stack([
        outs[0] + outs[1] + outs[2] + outs[3],
        outs[4] + outs[5] + outs[6] + outs[7],
    ]).astype(np.float32)
    full += np.asarray(b_o, np.float32)
    return full



# revision 50
# speedup vs baseline: 1.0019x; 1.0019x over previous
"""Trainium2 Bass kernel for a diffusers-style cross-attention block.

Problem (hardcoded shapes):
    hidden_states         [2, 2048, 1280] f32
    encoder_hidden_states [2, 2048, 1024] f32
    Wq [1280, 1280]  Wk/Wv [1024, 1280]  Wo [1280, 1280]  b_o [1280]  (all f32)
    out = softmax((x Wq) (enc Wk)^T / 8) (enc Wv) Wo + b_o      (20 heads x 64)

Sharding across 8 NeuronCores: data-parallel on batch (2) x tensor-parallel on
heads (4 groups of 5 heads). Each core computes a partial output
[2048, 1280] = A_local @ Wo_rows for its 5 heads; the host sums the 4 partials
per batch element and adds the bias.

Per-core layout: the host passes TRANSPOSED activations (x^T, enc^T, bf16), so
Q^T and K^T come straight out of the projection matmuls, scores are computed
as S^T (kv on partitions, q on free) and exp runs on the scalar engine
PSUM->SBUF. The PV matmul uses lhsT=P^T tiles (M=128 q rows) with rhs=V
blocks (N=65 incl. a ones column that emits the softmax denominator), so it
streams only 65 columns per kv-tile -- half the engine time of the M=65/N=512
formulation.  The resulting A [q, d] tiles are normalized (reciprocal of the
l column broadcast per-partition, fused into the PSUM->SBUF drain) and
transposed back to A^T for the output projection with DMA xbar transposes
(SBUF->SBUF, no PE or PSUM cost); the final round uses a PE transpose
instead, since the xbar round-trip would gate the last out-projection rows.

Scheduling: the PV matmuls run one iteration behind the score/exp stream
(lag-1 software pipeline) so the scalar engine never stalls at round
boundaries.  The V projection is split by head-group and interleaved into
round 0 of each attention stage, which balances the tensor engine against
the scalar engine's exp throughput in every round.  Stages run in order t=2
(single head 4), t=0, t=1: the out-projection is gated by the LAST stage's
A tiles, and a pair stage has twice the exp time per round under which to
hide the out-projection matmuls.  x^T lands q-block-major so stage t=2's
first q-block projection unblocks after 1/4 of the x^T bytes (its other
q-blocks stream in later as attention-round fills); the prologue projects
kt2's first kv-block plus all of Kt0 k-outer, each kstep firing as its
enc^T chunk lands, putting the first exp ~23us in instead of ~35us.
"""

import numpy as np
import ml_dtypes
from contextlib import ExitStack

S = 2048          # seq len (q and kv)
C = 1280          # hidden
CC = 1024         # encoder hidden
HG = 5            # heads per core
D = 64            # head dim
HD = HG * D       # 320
VW = D + 1        # V columns incl. ones column
CK = C // 128     # 10
CCK = CC // 128   # 8
NKV = S // 128    # 16
NQ = S // 512     # 4

_CACHED = {}

# scheduling knobs (sweepable; defaults = best known)
CONFIG = {
    "phat_bufs": 12,
    "osb_bufs": 4,
    "small_bufs": 2,
    "f1_group": 3, "f2_group": 2, "og_group": 3,
    "an_bufs": 6,
    "rate0": 1.3, "rate1": 1.0, "rate2": 1.5, "fill_start": 4,
}


def _emit(ctx, tc, xT, encT, wq, wk, wv, wo, out):
    from concourse import mybir, masks

    nc = tc.nc
    bf16, f32 = mybir.dt.bfloat16, mybir.dt.float32
    Exp = mybir.ActivationFunctionType.Exp

    const = ctx.enter_context(tc.tile_pool(name="const", bufs=1))
    acts = ctx.enter_context(tc.tile_pool(name="acts", bufs=1))
    small = ctx.enter_context(tc.tile_pool(name="small", bufs=CONFIG["small_bufs"]))
    osb_pool = ctx.enter_context(tc.tile_pool(name="osb", bufs=CONFIG["osb_bufs"]))
    phat_pool = ctx.enter_context(tc.tile_pool(name="phat", bufs=CONFIG["phat_bufs"]))
    an_pool = ctx.enter_context(tc.tile_pool(name="an", bufs=CONFIG["an_bufs"]))
    psum = ctx.enter_context(tc.tile_pool(name="psum", bufs=2, space="PSUM"))

    # ---- DMA in, critical-path order. enc^T and x^T land one 128-row
    # contraction chunk at a time so the k-outer prologue projections can
    # start each kstep as soon as its chunk arrives. wv's head-4 columns get
    # a tiny early DMA (stage t=2's V projection runs ~2us into round 0);
    # the rest of wv follows x^T.
    wk_sb = const.tile([128, CCK * HD], bf16, tag="wk")
    nc.sync.dma_start(wk_sb[:], wk.rearrange("(k p) d -> p k d", p=128))
    encT_big = acts.tile([128, CCK * S], bf16, tag="encT")
    for k in range(CCK):
        nc.sync.dma_start(encT_big[:, k * S:(k + 1) * S],
                          encT[k * 128:(k + 1) * 128, :])
    encT_sb = [encT_big[:, k * S:(k + 1) * S] for k in range(CCK)]
    wq_sb = const.tile([128, CK * HD], bf16, tag="wq")
    nc.sync.dma_start(wq_sb[:], wq.rearrange("(k p) d -> p k d", p=128))
    wv_sb = const.tile([128, CCK * HD], bf16, tag="wv")
    wv_pkd = wv.rearrange("(k p) d -> p k d", p=128)
    wv_sb_pkd = wv_sb.rearrange("p (k d) -> p k d", k=CCK)
    nc.sync.dma_start(wv_sb_pkd[:, :, 4 * D:5 * D], wv_pkd[:, :, 4 * D:5 * D])
    # x^T lands q-block-major (j-outer, k-pair chunks): stage t=2's round 0
    # only reads q-block 0, so its Q projection unblocks after 1/4 of x^T;
    # j=1..3 projections run as fills inside the attention rounds.
    xT_big = acts.tile([128, CK * S], bf16, tag="xT")
    xT_kv = xT_big.rearrange("p (k s) -> p k s", k=CK)
    for j in range(NQ):
        for kp in range(0, CK, 2):
            nc.sync.dma_start(
                xT_kv[:, kp:kp + 2, j * 512:(j + 1) * 512],
                xT[kp * 128:(kp + 2) * 128, j * 512:(j + 1) * 512]
                .rearrange("(k p) s -> p k s", p=128),
            )
    xT_sb = [xT_big[:, k * S:(k + 1) * S] for k in range(CK)]
    nc.sync.dma_start(wv_sb_pkd[:, :, 0:4 * D], wv_pkd[:, :, 0:4 * D])
    wo_sb = []
    for t in range(3):
        K = 128 if t < 2 else 64
        w = const.tile([128, C], bf16, tag=f"wo{t}", name=f"wo{t}")
        nc.sync.dma_start(w[:K, :], wo[t * 128:t * 128 + K, :])
        wo_sb.append(w)

    ident = const.tile([128, 128], bf16, tag="ident")
    masks.make_identity(nc, ident[:])

    # persistent intermediates (head pairs packed into 128 partitions)
    qt_sb = [acts.tile([128, S], bf16, tag=f"qt{t}", name=f"qt{t}") for t in range(3)]
    kt_sb = [acts.tile([128, S], bf16, tag=f"kt{t}", name=f"kt{t}") for t in range(3)]
    at_sb = [acts.tile([128, S], bf16, tag=f"at{t}", name=f"at{t}") for t in range(3)]
    v_sb = acts.tile([128, NKV * HG * VW], bf16, tag="v")
    nc.vector.memset(v_sb[:], 1.0)  # ones columns; V blocks overwritten below

    def proj_qk_steps(w_sb, src_sb, nk, dst, t, group):
        """Generator: emits the Q/K projection for tile t in ~group-MM slices."""
        M = 128 if t < 2 else 64
        for j in range(NQ):
            ps = psum.tile([128, 512], f32, tag=("s" if j % 2 == 0 else "fill"),
                           name="ps", bufs=2)
            for k in range(nk):
                nc.tensor.matmul(
                    ps[:M, :],
                    lhsT=w_sb[:, k * HD + t * 128: k * HD + t * 128 + M],
                    rhs=src_sb[k][:, j * 512:(j + 1) * 512],
                    start=(k == 0), stop=(k == nk - 1),
                )
                if (k + 1) % group == 0:
                    yield
            nc.vector.tensor_copy(dst[:M, j * 512:(j + 1) * 512], ps[:M, :])
            yield

    def proj_qk(w_sb, src_sb, nk, dst, t):
        for _ in proj_qk_steps(w_sb, src_sb, nk, dst, t, group=999):
            pass

    def proj_v_tile(i, h0, nhv):
        # V projection for heads h0..h0+nhv-1 of kv-tile i (ones cols kept)
        ps = psum.tile([128, 512], f32, tag="fill", name="ps", bufs=2)
        for k in range(CCK):
            nc.tensor.matmul(
                ps[:, :nhv * D],
                lhsT=encT_sb[k][:, i * 128:(i + 1) * 128],
                rhs=wv_sb[:, k * HD + h0 * D: k * HD + (h0 + nhv) * D],
                start=(k == 0), stop=(k == CCK - 1),
            )
        for h in range(h0, h0 + nhv):
            nc.vector.tensor_copy(
                v_sb[:, (i * HG + h) * VW: (i * HG + h) * VW + D],
                ps[:, (h - h0) * D:(h - h0 + 1) * D],
            )

    def proj_qk_fill_steps(w_sb, src_sb, nk, dst, t, group, tag="fill",
                           js=tuple(range(NQ))):
        M = 128 if t < 2 else 64
        for j in js:
            ps = psum.tile([128, 512], f32, tag=tag, name="ps", bufs=2)
            for k in range(nk):
                nc.tensor.matmul(
                    ps[:M, :],
                    lhsT=w_sb[:, k * HD + t * 128: k * HD + t * 128 + M],
                    rhs=src_sb[k][:, j * 512:(j + 1) * 512],
                    start=(k == 0), stop=(k == nk - 1),
                )
                if (k + 1) % group == 0:
                    yield
            nc.vector.tensor_copy(dst[:M, j * 512:(j + 1) * 512], ps[:M, :])
            yield

    def proj_k_outer_steps(w_sb, src_sb, nk, dst, t, js=tuple(range(NQ)),
                           tag=None, tags=None):
        """k-outer projection over the q-blocks in js: one kstep covers all
        of them, so kstep k can start as soon as src chunk k's DMA lands.
        Yields per kstep."""
        M = 128 if t < 2 else 64
        if tags is None:
            tags = [(tag or ("s" if ji % 2 == 0 else "fill"))
                    for ji in range(len(js))]
        tiles = {j: psum.tile([128, 512], f32, tag=tags[ji],
                              name="pko", bufs=2) for ji, j in enumerate(js)}
        for k in range(nk):
            for j in js:
                nc.tensor.matmul(
                    tiles[j][:M, :],
                    lhsT=w_sb[:, k * HD + t * 128: k * HD + t * 128 + M],
                    rhs=src_sb[k][:, j * 512:(j + 1) * 512],
                    start=(k == 0), stop=(k == nk - 1),
                )
                if k == nk - 1:
                    # copy each j the moment its chain stops -- j=0's copy
                    # gates the first scores of the following stage
                    nc.vector.tensor_copy(dst[:M, j * 512:(j + 1) * 512],
                                          tiles[j][:M, :])
            yield

    def outproj_steps(ms, group, tail=False):
        for m in ms:
            osb = osb_pool.tile([128, C], bf16, tag="osb", name="osb")
            cnt = 0
            for ci, c0 in enumerate(range(0, C, 512)):
                cn = min(512, C - c0)
                ps = psum.tile([128, 512], f32,
                               tag=("s" if tail and (m * 3 + ci) % 2 else "fill"),
                               name="ops", bufs=2)
                # t=1 (the last stage's rows, the gating input) contracts
                # last so the other partials can run before at[1] lands
                for ti, t in enumerate((0, 2, 1)):
                    K = 128 if t < 2 else 64
                    nc.tensor.matmul(
                        ps[:, :cn],
                        lhsT=at_sb[t][:K, m * 128:(m + 1) * 128],
                        rhs=wo_sb[t][:K, c0:c0 + cn],
                        start=(ti == 0), stop=(ti == 2),
                    )
                    cnt += 1
                    if cnt % group == 0:
                        yield
                # in the tail the scalar engine is idle -- alternate the
                # psum->sbuf drains across it and DVE
                if tail and (m + ci) % 2:
                    nc.scalar.copy(osb[:, c0:c0 + cn], ps[:, :cn])
                else:
                    nc.vector.tensor_copy(osb[:, c0:c0 + cn], ps[:, :cn])
                nc.sync.dma_start(out[m * 128:(m + 1) * 128, c0:c0 + cn],
                                  osb[:, c0:c0 + cn])
            yield

    def attention(t, v_interleave=False, fills=(None, None, None, None),
                  fill_rate=1.0, fill_start=0, tail_stage=False):
        """Four 512-wide q-block rounds; fills[r] is a generator driven during
        round r (must only read data produced in rounds < r). Pair rounds pack
        both heads side-by-side in one [128,1024] score tile -> single exp.
        fill_rate may be fractional (yields per iteration, spread evenly);
        fill_start delays drives by that many iterations -- needed when the
        fill reads at_sb written by the deferred transpose flush.

        The PV matmuls run one iteration behind the score/exp stream (lag-1
        software pipeline) so the next round's scores are already queued on
        the PE when it waits for the current exp -- the scalar engine never
        stalls at round boundaries."""
        heads = (2 * t, 2 * t + 1) if t < 2 else (4,)
        nh = len(heads)
        h0v = heads[0]
        M = 128 if nh == 2 else 64
        istep = 2 // nh
        state = {"pv": None, "prev": None}

        def norm_round(jb_, pvd):
            # normalize A rows by the denominator column, packing head pairs
            # side-by-side; recips first, then muls qt-major so an[qt]
            # completes early in the DVE stream. The A -> A^T transpose is a
            # DMA xbar transpose (SBUF->SBUF, ~14ns per 16x128 tile) issued
            # from the SP queue: no PE time, no PSUM banks. For the
            # single-head stage the unused half of an is zeroed once so the
            # full 128-wide tile is transposable (at rows 64+ are unused).
            an = [an_pool.tile([128, 128], bf16, tag="an", name="an",
                               bufs=CONFIG["an_bufs"]) for _ in range(4)]
            linvs = {}
            for h in heads:
                linv = small.tile([128, 4], f32, tag="linv", name="linv")
                nc.vector.reciprocal(linv[:], pvd[h][:, D::VW])
                linvs[h] = linv
            tail_round = tail_stage and jb_ == NQ - 1
            for qt in range(4):
                if nh == 1:
                    nc.vector.memset(an[qt][:, 64:128], 0.0)
                for h in heads:
                    rb0 = (h % 2) * 64
                    if tail_round and h % 2:
                        # scalar engine is idle after the last exp; splitting
                        # the normalization muls halves the at-ready latency
                        nc.scalar.activation(
                            an[qt][:, rb0:rb0 + 64],
                            pvd[h][:, qt * VW: qt * VW + D],
                            mybir.ActivationFunctionType.Copy,
                            scale=linvs[h][:, qt:qt + 1],
                        )
                    else:
                        nc.vector.tensor_scalar_mul(
                            an[qt][:, rb0:rb0 + 64],
                            pvd[h][:, qt * VW: qt * VW + D],
                            linvs[h][:, qt:qt + 1],
                        )
                if tail_round:
                    # tail: the score pipeline is done, PSUM banks are free
                    # and the xbar round-trip (~2.5us) would gate the last
                    # out-projection rows -- use the PE transpose instead
                    tp = psum.tile([128, 128], bf16, tag="s", name="tp", bufs=2)
                    nc.tensor.transpose(tp[:, :], an[qt][:, :], ident[:])
                    nc.vector.tensor_copy(
                        at_sb[t][:, jb_ * 512 + qt * 128: jb_ * 512 + (qt + 1) * 128],
                        tp[:, :],
                    )
                else:
                    nc.sync.dma_start_transpose(
                        at_sb[t][:, jb_ * 512 + qt * 128: jb_ * 512 + (qt + 1) * 128],
                        an[qt][:, :],
                    )

        def emit_prev_pv():
            # PV in A-layout: lhsT = P^T q-tile (M=128 q rows), rhs = V
            # block (N=65 cols incl. the denominator ones column). PSUM
            # zeroing is per 2KB bank, so the whole pv tile is one
            # accumulation group: start only on the first touch of the
            # bank, stop on the last; intermediate first-touches overwrite
            # their still-pending-zero bytes.
            if state["prev"] is None:
                return
            ph_, i0_, jb_ = state["prev"]
            state["prev"] = None
            if i0_ == 0:
                state["pv"] = {
                    h: psum.tile([128, 4 * VW], f32, tag="pv", name="pv", bufs=2)
                    for h in heads
                }
            pvd = state["pv"]
            for sx in range(2):
                h = heads[sx % nh]
                i = i0_ + sx // nh
                for qt in range(4):
                    nc.tensor.matmul(
                        pvd[h][:, qt * VW:(qt + 1) * VW],
                        lhsT=ph_[:, sx * 512 + qt * 128: sx * 512 + (qt + 1) * 128],
                        rhs=v_sb[:, (i * HG + h) * VW: (i * HG + h + 1) * VW],
                        start=(i == 0 and qt == 0),
                        stop=(i == NKV - 1 and qt == 3),
                    )
            if i0_ == NKV - istep:
                norm_round(jb_, pvd)

        nits = NKV // istep
        for jb in range(NQ):
            fill = fills[jb]
            acc = 0.0
            # pair rounds pack (headA | headB) per kv-tile; single-head
            # rounds pack (kv-tile i | kv-tile i+1) -- one exp per 1024 cols
            for it, i0 in enumerate(range(0, NKV, istep)):
                sps = psum.tile([128, 1024], f32, tag="s", name="sps", bufs=2)
                for sx in range(2):
                    h = heads[sx % nh]
                    i = i0 + sx // nh
                    rb0 = (h % 2) * 64
                    nc.tensor.matmul(
                        sps[:, sx * 512:(sx + 1) * 512],
                        lhsT=kt_sb[t][rb0:rb0 + 64, i * 128:(i + 1) * 128],
                        rhs=qt_sb[t][rb0:rb0 + 64, jb * 512:(jb + 1) * 512],
                        start=True, stop=True,
                    )
                ph = phat_pool.tile([128, 1024], bf16, tag="ph", name="ph")
                nc.scalar.activation(ph[:], sps[:], Exp, scale=0.125)
                emit_prev_pv()
                if v_interleave and jb == 0:
                    for i in range(i0, i0 + istep):
                        proj_v_tile(i, h0v, nh)
                if fill is not None and fill_start <= it < nits - 2:
                    acc += fill_rate
                    while acc >= 1.0:
                        next(fill, None)
                        acc -= 1.0
                state["prev"] = (ph, i0, jb)
        emit_prev_pv()

    def chain(*gens):
        for g in gens:
            yield from g

    def drain(g):
        for _ in g:
            pass

    # prologue pass A (paced by the enc^T chunk DMAs): kt2's first kv-block
    # -- the only one the first two score iterations read -- plus ALL of Kt0,
    # zipped kstep-for-kstep on the same chunk deps across the s/pv/fill psum
    # tags. kt2's remaining kv-blocks follow during the x^T j0 DMA window,
    # then Q-t2 j0 streams in chunk-chained with nothing in its way.
    genKj0 = proj_k_outer_steps(wk_sb, encT_sb, CCK, kt_sb[2], 2,
                                js=(0,), tags=("s",))
    genk0a = proj_k_outer_steps(wk_sb, encT_sb, CCK, kt_sb[0], 0,
                                js=(0, 1), tags=("pv", "pv"))
    genk0b = proj_k_outer_steps(wk_sb, encT_sb, CCK, kt_sb[0], 0,
                                js=(2, 3), tags=("fill", "fill"))
    for _ in genKj0:
        next(genk0a, None)
        next(genk0b, None)
    drain(genk0a)
    drain(genk0b)
    drain(proj_k_outer_steps(wk_sb, encT_sb, CCK, kt_sb[2], 2,
                             js=(1, 2, 3), tags=("s", "fill", "s")))
    drain(proj_qk_fill_steps(wq_sb, xT_sb, CK, qt_sb[2], 2,
                             CONFIG["f1_group"], tag="s", js=(0,)))

    fill1 = chain(
        proj_qk_fill_steps(wq_sb, xT_sb, CK, qt_sb[2], 2, CONFIG["f1_group"], js=(1,)),
        proj_qk_fill_steps(wq_sb, xT_sb, CK, qt_sb[2], 2, CONFIG["f1_group"], js=(2,)),
        proj_qk_fill_steps(wq_sb, xT_sb, CK, qt_sb[2], 2, CONFIG["f1_group"], js=(3,)),
        proj_qk_fill_steps(wq_sb, xT_sb, CK, qt_sb[0], 0, CONFIG["f1_group"]),
    )
    attention(2, v_interleave=True, fills=(fill1, fill1, fill1, fill1),
              fill_rate=CONFIG["rate0"], fill_start=2)
    fill2 = chain(fill1,
                  proj_qk_fill_steps(wk_sb, encT_sb, CCK, kt_sb[1], 1, CONFIG["f2_group"]),
                  proj_qk_fill_steps(wq_sb, xT_sb, CK, qt_sb[1], 1, CONFIG["f2_group"]))
    attention(0, v_interleave=True, fills=(None, fill2, fill2, fill2),
              fill_rate=CONFIG["rate1"])
    drain(fill2)
    # out-projection row-group k (m = 4k..4k+3) reads at columns produced by
    # round k of every stage; the last stage (t=1) writes them at iteration 0
    # of round k+1 via the norm + xbar-transpose path, so the og chain may
    # only be driven from fill_start iterations in, and at a rate that keeps
    # cumulative drives by end of round r within r*20 yields (og gating).
    og = [outproj_steps(range(4 * k, 4 * k + 4), CONFIG["og_group"]) for k in range(3)]
    ogc = chain(*og)
    attention(1, v_interleave=True, fills=(None, ogc, ogc, ogc),
              fill_rate=CONFIG["rate2"], fill_start=CONFIG["fill_start"],
              tail_stage=True)
    drain(ogc)
    drain(outproj_steps(range(12, NKV), 999, tail=True))


def build():
    if "nc" in _CACHED:
        return _CACHED["nc"]
    import concourse.tile as tile
    from concourse import bacc, mybir

    bf16, f32 = mybir.dt.bfloat16, mybir.dt.float32
    nc = bacc.Bacc("TRN2", target_bir_lowering=False, debug=False)
    xT = nc.dram_tensor("xT", [C, S], bf16, kind="ExternalInput").ap()
    encT = nc.dram_tensor("encT", [CC, S], bf16, kind="ExternalInput").ap()
    wq = nc.dram_tensor("wq", [C, HD], bf16, kind="ExternalInput").ap()
    wk = nc.dram_tensor("wk", [CC, HD], bf16, kind="ExternalInput").ap()
    wv = nc.dram_tensor("wv", [CC, HD], bf16, kind="ExternalInput").ap()
    wo = nc.dram_tensor("wo", [HD, C], bf16, kind="ExternalInput").ap()
    out = nc.dram_tensor("out", [S, C], bf16, kind="ExternalOutput").ap()

    with tile.TileContext(nc) as tc:
        with ExitStack() as ctx:
            _emit(ctx, tc, xT, encT, wq, wk, wv, wo, out)
    nc.compile()
    _CACHED["nc"] = nc
    return nc


def make_in_maps(hidden_states, encoder_hidden_states, Wq, Wk, Wv, Wo):
    bf = ml_dtypes.bfloat16
    in_maps = []
    xTs = [np.ascontiguousarray(hidden_states[b].T).astype(bf) for b in range(2)]
    encTs = [np.ascontiguousarray(encoder_hidden_states[b].T).astype(bf) for b in range(2)]
    for core in range(8):
        b, g = divmod(core, 4)
        cols = slice(g * HD, (g + 1) * HD)
        in_maps.append({
            "xT": xTs[b],
            "encT": encTs[b],
            "wq": np.ascontiguousarray(Wq[:, cols]).astype(bf),
            "wk": np.ascontiguousarray(Wk[:, cols]).astype(bf),
            "wv": np.ascontiguousarray(Wv[:, cols]).astype(bf),
            "wo": np.ascontiguousarray(Wo[cols, :]).astype(bf),
        })
    return in_maps


def kernel(hidden_states, encoder_hidden_states, Wq, Wk, Wv, Wo, b_o):
    from concourse.bass_utils import run_bass_kernel_spmd

    nc = build()
    in_maps = make_in_maps(hidden_states, encoder_hidden_states, Wq, Wk, Wv, Wo)
    res = run_bass_kernel_spmd(nc, in_maps, core_ids=list(range(8)))
    # partials come back bf16 (halves the output DMA); sum in f32 on host
    outs = [np.asarray(res.results[c]["out"], np.float32) for c in range(8)]
    full = np.stack([
        outs[0] + outs[1] + outs[2] + outs[3],
        outs[4] + outs[5] + outs[6] + outs[7],
    ])
    full += np.asarray(b_o, np.float32)
    return full

